# revision 44
# baseline (speedup 1.0000x reference)
"""Trainium2 Bass kernel for nn_ClusterModel (MoE routing + segment pooling).

Model:
  xg = x[group_indices]                         # [4, N/4, 128] per-group gather
  h  = relu(xg @ W1[g] + b1[g])                 # [4, N/4, 1024]
  og = h @ W2[g] + b2[g]                        # [4, N/4, 512]
  new_feat = scatter(og) back to node order     # [N, 512]
  emb = segment_max(new_feat, fine clusters)    # [8192, 512]  (16 nodes/cluster)
  normed = InstanceNorm per coarse graph        # [8192, 512]  (256 clusters/graph)
  logits = normed @ w_out + b_out               # [8192, 16]

Sharding: 8 cores, each takes N/8 consecutive nodes = 4 coarse graphs.
All segment reductions are core-local -> zero collectives.

v3 design (vs v2):
  * bf16 GEMMs (full PE rate); og scratch is ONE flat DRAM tensor
    [2 sentinels + NG*GCAP rows, H] (row ids fit int16), written with a
    single batched DMA per chunk -- no per-window margin double-writes.
  * cluster-block gathers are prepare_only SWDGE preps on 2 queues,
    descriptor-gen (8ns/desc on Pool) runs one chunk-set EARLY, and
    trigger_dma fires at the readiness boundary with explicit sync deps
    on that set's og-write DMAs.  Max-tree waits on the baked DMA sems.
  * first-use-ordered weight loads (xt chunk 0 first, then w1[g0],
    w2[g0] per-kt slices; later groups stream during group g-1).
  * og PSUM evacuation in H-halves on ACT+DVE concurrently.
  * depth-2 software pipeline: MM_h(two units ahead) issues before the
    og MMs of the current k-tile so the relu never stalls the PE.
  * Rsqrt (act-table set 14, shared with Relu/Copy/Square) for the
    instance-norm -> no act-table reload.
"""

import numpy as np
from contextlib import ExitStack

import jax
import ml_dtypes
import concourse.bass as bass
import concourse.tile as tile
from concourse import bacc, mybir
from concourse import bass2jax

F32 = mybir.dt.float32
F32R = mybir.dt.float32r
BF16 = mybir.dt.bfloat16
I16 = mybir.dt.int16
AF = mybir.ActivationFunctionType
ALU = mybir.AluOpType

# Problem constants (hardcoded per contest contract)
N = 131072
D = 128
KEXP = 1024
H = 512
NG = 4
F_SEG = 8192
G_SEG = 32
C_CLS = 16
EPS = 1e-5
NCORES = 8
P = 128
NEG = -3.0e38

_PROGRAM_CACHE: dict = {}


# ----------------------------------------------------------------------------
# Device program
# ----------------------------------------------------------------------------

def _build_program(GCAP: int, CCAP: int, MCAP: int, M: int, zero_b2: bool,
                   phases: int = 5):
    """Build the SPMD Bass program.

    GCAP: padded rows per (core, group), multiple of 128
    CCAP: padded clusters per (core, graph), multiple of 128
    MCAP: padded members per cluster
    M:    window margin rows (grid is 512-spaced in group-row space)
    zero_b2: skip the og bias add (b2 == 0)
    """
    GPC = G_SEG // NCORES          # graphs per core = 4
    SLOTS = GPC * CCAP             # cluster slots per core
    NBLK = SLOTS // P              # cluster blocks / windows (128 clusters)
    BPG = CCAP // P                # blocks per graph
    KT = KEXP // P                 # 8 k-tiles in layer 2
    FT = H // P                    # 4 feature tiles of H
    OGROWS = 2 + NG * GCAP         # flat og tensor rows (2 sentinels)
    IDXW = MCAP * P // 16          # idx cols per window

    # chunk schedule in group-row space: 512-row chunks + remainder
    widths = []
    r = 0
    while r < GCAP:
        w = min(512, GCAP - r)
        widths.append(w)
        r += w
    NCHUNK = len(widths)
    RTOT = NG * GCAP

    # window readiness: og_w complete after chunk-set covering 512(w+1)+M
    ready = []
    for w in range(NBLK):
        need = min(GCAP, 512 * (w + 1) + M)
        acc = 0
        for j, cw in enumerate(widths):
            acc += cw
            if acc >= need:
                ready.append(j)
                break
    assert len(ready) == NBLK

    NQ = 2                         # SWDGE queues used for gathers
    assert MCAP % NQ == 0
    nc = bacc.Bacc("TRN2", target_bir_lowering=False, debug=False,
                   num_devices=NCORES, num_swdge_queues=NQ)

    xt_ap = nc.dram_tensor("xt", [P, RTOT], BF16, kind="ExternalInput").ap()
    w1_ap = nc.dram_tensor("w1", [P, NG, KEXP], BF16, kind="ExternalInput").ap()
    w2_ap = nc.dram_tensor("w2", [P, NG, KT, H], BF16, kind="ExternalInput").ap()
    b1_ap = nc.dram_tensor("b1s", [P, NG * KT], F32, kind="ExternalInput").ap()
    b2_ap = nc.dram_tensor("b2r", [P, NG, H], F32, kind="ExternalInput").ap()
    wo_ap = nc.dram_tensor("wout", [P, FT, C_CLS], F32, kind="ExternalInput").ap()
    bo_ap = nc.dram_tensor("bout", [C_CLS, 1], F32, kind="ExternalInput").ap()
    ic_ap = nc.dram_tensor("invc", [P, GPC], F32, kind="ExternalInput").ap()
    gi_ap = nc.dram_tensor("gidx", [P, NBLK * IDXW], I16,
                           kind="ExternalInput").ap()
    id_ap = nc.dram_tensor("ident", [P, P], BF16, kind="ExternalInput").ap()
    og_ap = nc.dram_tensor("ogs", [OGROWS, H], BF16).ap()
    lo_ap = nc.dram_tensor("logt", [C_CLS, SLOTS], F32, kind="ExternalOutput").ap()
    dbg_og_ap = dbg_emb_ap = None
    if phases <= 1:
        dbg_og_ap = nc.dram_tensor("dbg_og", [OGROWS, H], BF16,
                                   kind="ExternalOutput").ap()
    elif phases <= 3:
        dbg_emb_ap = nc.dram_tensor("dbg_emb", [P, NBLK, H], BF16,
                                    kind="ExternalOutput").ap()

    with tile.TileContext(nc) as tc, ExitStack() as ctx:
        cst = ctx.enter_context(tc.tile_pool(name="cst", bufs=1))

        # --- resident constants -------------------------------------------
        # w1/w2 load in first-use order: group 0's slices land before the
        # pipeline needs them, later groups stream in during group g-1's
        # first chunk.  The rest defer into sync-queue gaps.  (The xt DMA
        # for chunk 0 is emitted even earlier, by the first emit_mm_h.)
        w1_sb = cst.tile([P, NG, KEXP], BF16)
        b1_sb = cst.tile([P, NG * KT], F32)
        w2_sb = cst.tile([P, NG, KT, H], BF16)

        def emit_consts_w1g0():
            nc.sync.dma_start(out=w1_sb[:, 0, :], in_=w1_ap[:, 0, :])

        def emit_consts_first():
            nc.sync.dma_start(out=b1_sb[:], in_=b1_ap[:])
            for _kt in range(KT):
                nc.sync.dma_start(out=w2_sb[:, 0, _kt, :],
                                  in_=w2_ap[:, 0, _kt, :])
        b2_sb = cst.tile([P, NG, H], F32)
        wo_sb = cst.tile([P, FT, C_CLS], F32R)
        wo_raw = cst.tile([P, FT, C_CLS], F32)
        bo_sb = cst.tile([C_CLS, 1], F32)
        ic_sb = cst.tile([P, GPC], F32)
        gi_sb = cst.tile([P, NBLK * IDXW], I16)
        ident = cst.tile([P, P], BF16)
        sent0 = cst.tile([1, H], BF16)
        sent1 = cst.tile([1, H], BF16)

        def emit_consts_w2(g):
            nc.sync.dma_start(out=w2_sb[:, g, :, :], in_=w2_ap[:, g, :, :])
            if not zero_b2:
                nc.sync.dma_start(out=b2_sb[:, g, :], in_=b2_ap[:, g, :])

        def emit_consts_late():
            nc.sync.dma_start(out=wo_raw[:], in_=wo_ap[:])
            nc.vector.tensor_copy(wo_sb[:], wo_raw[:])
            nc.sync.dma_start(out=bo_sb[:], in_=bo_ap[:])
            nc.sync.dma_start(out=ic_sb[:], in_=ic_ap[:])
            nc.sync.dma_start(out=gi_sb[:], in_=gi_ap[:])
            nc.sync.dma_start(out=ident[:], in_=id_ap[:])
            nc.vector.memset(sent0[:], 0.0)
            nc.vector.memset(sent1[:], NEG)
            nc.sync.dma_start(out=og_ap[0:1, :], in_=sent0[:])
            nc.sync.dma_start(out=og_ap[1:2, :], in_=sent1[:])

        emb_sb = cst.tile([P, NBLK, H], BF16)   # pooled embeddings

        # --- phase 1 + interleaved gather/max ------------------------------
        p1ctx = ExitStack()
        gsb = p1ctx.enter_context(tc.tile_pool(name="g_xt", bufs=3))
        ght = p1ctx.enter_context(tc.tile_pool(name="g_ht", bufs=4))
        gog = p1ctx.enter_context(tc.tile_pool(name="g_og", bufs=6))
        gph = p1ctx.enter_context(tc.tile_pool(name="g_ph", bufs=3, space="PSUM"))
        gpo = p1ctx.enter_context(tc.tile_pool(name="g_po", bufs=4, space="PSUM"))
        p2 = p1ctx.enter_context(tc.tile_pool(name="p2", bufs=3))
        p2t = p1ctx.enter_context(tc.tile_pool(name="p2t", bufs=2))
        p3ps = p1ctx.enter_context(tc.tile_pool(name="p3ps", bufs=1,
                                                space="PSUM"))

        # chunk list in (j-outer, g-inner) order
        chunks = []
        for j in range(NCHUNK):
            for g in range(NG):
                chunks.append((j, g))

        def chunk_units(ci):
            j, g = chunks[ci]
            return [(ci, kt) for kt in range(KT)]

        units = []
        for ci in range(len(chunks)):
            units.append(chunk_units(ci))
        flat_units = [u for cu in units for u in cu]

        # per-chunk state
        xt_tiles = {}
        h_tiles = {}
        og_tiles = {}
        og_write_insts = {}     # chunk-set j -> og window-write DMA insts
        self_fence = [None]     # most recent DVE relu op (scheduling fence)
        pe_fence = [None]       # most recent og matmul (scheduling fence)
        gather_sems = [nc.alloc_semaphore(f"gsem{w}_{q}")
                       for w in range(NBLK) for q in range(NQ)]

        def col0_of(ci):
            j, g = chunks[ci]
            return g * GCAP + sum(widths[:j]), widths[j], j, g

        def emit_mm_h(ci, kt):
            col0, width, j, g = col0_of(ci)
            if ci not in xt_tiles:
                xt_sb = gsb.tile([P, 512], BF16, tag="xt")
                nc.sync.dma_start(out=xt_sb[:, :width],
                                  in_=xt_ap[:, col0:col0 + width])
                xt_tiles[ci] = xt_sb
            h_ps = gph.tile([P, 512], F32, tag="h")
            nc.tensor.matmul(h_ps[:, :width],
                             w1_sb[:, g, kt * P:(kt + 1) * P],
                             xt_tiles[ci][:, :width], start=True, stop=True)
            h_tiles[(ci, kt)] = h_ps

        def emit_relu(ci, kt):
            col0, width, j, g = col0_of(ci)
            h_ps = h_tiles.pop((ci, kt))
            ht_sb = ght.tile([P, 512], BF16, tag="ht")
            if kt % 2 == 0:
                nc.scalar.activation(
                    ht_sb[:, :width], h_ps[:, :width], AF.Relu,
                    bias=b1_sb[:, g * KT + kt:g * KT + kt + 1])
            else:
                op = nc.vector.tensor_scalar(
                    ht_sb[:, :width], h_ps[:, :width],
                    b1_sb[:, g * KT + kt:g * KT + kt + 1], 0.0,
                    op0=ALU.add, op1=ALU.max)
                self_fence[0] = op
            return ht_sb

        def emit_og_mms(ci, kt, ht_sb):
            col0, width, j, g = col0_of(ci)
            ns = width // P
            if kt == 0:
                og_tiles[ci] = [gpo.tile([P, H], F32, tag="og",
                                         name=f"og_ps{ci}_{s}")
                                for s in range(ns)]
            for s in range(ns):
                pe_fence[0] = nc.tensor.matmul(
                    og_tiles[ci][s][:],
                    ht_sb[:, s * P:(s + 1) * P],
                    w2_sb[:, g, kt, :],
                    start=(kt == 0), stop=(kt == KT - 1))
            if kt == KT - 1:
                emit_og_evac(ci)

        def emit_og_evac(ci):
            """Evacuate og PSUM tiles into one SBUF tile + window writes."""
            col0, width, j, g = col0_of(ci)
            r0 = sum(widths[:j])            # group-row of chunk start
            ns = width // P
            og_sb = gog.tile([P, 4, H], BF16, tag="og")
            HH = H // 2
            for s in range(ns):
                # H-halves on ACT + DVE concurrently: the PSUM bank frees
                # as soon as both halves land, so the next chunk's og
                # matmuls get their banks back quickly.
                if zero_b2:
                    nc.scalar.activation(og_sb[:, s, :HH],
                                         og_tiles[ci][s][:, :HH], AF.Copy)
                    nc.vector.tensor_copy(og_sb[:, s, HH:],
                                          og_tiles[ci][s][:, HH:])
                else:
                    nc.scalar.activation(
                        og_sb[:, s, :HH], og_tiles[ci][s][:, :HH], AF.Copy,
                        bias=b2_sb[:, g, :HH])
                    nc.vector.tensor_tensor(
                        out=og_sb[:, s, HH:], in0=og_tiles[ci][s][:, HH:],
                        in1=b2_sb[:, g, HH:], op=ALU.add)
            dst = 2 + g * GCAP + r0
            out3 = og_ap[dst:dst + width, :].rearrange(
                "(s p) h -> p s h", p=P)
            op = nc.sync.dma_start(out=out3, in_=og_sb[:, :ns, :])
            og_write_insts.setdefault(j, []).append(op)
            del og_tiles[ci]

        def emit_gather_prep(w):
            """Emit window w's SWDGE descriptor generation (prepare_only),
            split across queues.  Desc-gen costs ~8ns/descriptor on the
            Pool engine, so it runs one chunk-set EARLY, fully overlapped
            with the GEMM; the DMA fires later via emit_gather_trigger."""
            gat = p2.tile([P, MCAP, H], BF16, tag="gat")
            half_idx = IDXW // NQ
            half_m = MCAP // NQ
            for q in range(NQ):
                idx_sl = gi_sb[:, w * IDXW + q * half_idx:
                               w * IDXW + (q + 1) * half_idx]
                nc.gpsimd.dma_gather(
                    gat[:, q * half_m:(q + 1) * half_m, :], og_ap[:],
                    idx_sl, MCAP * P // NQ, MCAP * P // NQ, H, elem_step=H,
                    single_packet=False, prepare_only=True,
                    sem=gather_sems[w * NQ + q], queue_num=q)
            return gat

        def emit_gather_trigger(w):
            """Fire window w's prepared DMAs.  The prep captured og-write
            deps only up to ITS emission (one boundary earlier), so add
            explicit sync deps on the readiness set's og-write DMAs."""
            from concourse.tile_rust import add_dep_helper
            for q in range(NQ):
                trig = nc.gpsimd.trigger_dma(count=1, queue_num=q)
                for dma in og_write_insts.get(ready[w], []):
                    add_dep_helper(trig.ins, dma.ins, sync=True,
                                   reason="gather after og writes of its set")

        def emit_tree(w, gat):
            from concourse.tile_rust import add_dep_helper
            # Tile attributes the prep's SBUF write to desc-gen, not DMA
            # completion, so wait for the baked DMA sems explicitly (each
            # prep's transfer bumps its sem by 16).  Pinned at the tree's
            # position in the DVE stream.
            prev = self_fence[0]
            waits = []
            for q in range(NQ):
                wop = nc.vector.wait_ge(gather_sems[w * NQ + q], 16)
                if prev is not None:
                    add_dep_helper(wop.ins, prev.ins, sync=False,
                                   reason="gather-sem wait in DVE order")
                prev = wop
                waits.append(wop)
            cur = gat
            m = MCAP
            first = True
            while m > 1:
                nxt_m = m // 2
                if nxt_m == 1:
                    op = nc.vector.tensor_tensor(
                        out=emb_sb[:, w, :], in0=cur[:, 0:1, :].opt({0}),
                        in1=cur[:, 1:2, :].opt({0}), op=ALU.max)
                else:
                    nxt = p2t.tile([P, nxt_m, H], BF16, tag=f"tm{nxt_m}")
                    op = nc.vector.tensor_tensor(
                        out=nxt[:], in0=cur[:, 0:nxt_m, :],
                        in1=cur[:, nxt_m:2 * nxt_m, :], op=ALU.max)
                    cur = nxt
                if first:
                    # pin the tree after the gather-sem waits (same DVE
                    # stream position; engine executes in order).
                    add_dep_helper(op.ins, waits[-1].ins, sync=False,
                                   reason="tree after gather-sem waits")
                first = False
                m = nxt_m
            # chain: the next tree's sem waits pin after this tree, so a
            # later window's (possibly blocking) wait can't jump ahead of
            # this window's remaining levels in the DVE queue.
            self_fence[0] = op

        # phase-3 transpose of one pooled window to feature-major
        embt = [cst.tile([P, GPC, CCAP], F32, tag=f"embt{f}", name=f"embt{f}")
                for f in range(FT)]

        def emit_phase3(w):
            from concourse.tile_rust import add_dep_helper
            gi_, bi_ = w // BPG, w % BPG
            for f in range(FT):
                tp = p3ps.tile([P, P], BF16, tag="tp", name=f"tp{w}_{f}")
                op = nc.tensor.transpose(
                    tp[:], emb_sb[:, w, f * P:(f + 1) * P], ident[:])
                if f == 0 and pe_fence[0] is not None:
                    add_dep_helper(op.ins, pe_fence[0].ins, sync=False,
                                   reason="transpose after current set's MMs")
                nc.scalar.activation(
                    embt[f][:, gi_, bi_ * P:(bi_ + 1) * P], tp[:], AF.Copy)

        # ---- main emission loop ----
        pending_gather = {}     # w -> gat tile (DMA triggered, tree pending)
        prepped = {}            # w -> gat tile (descriptors generated)
        treed = []
        p3done = 0
        assert all(r >= 1 for r in ready), "window ready before first boundary"
        nu = len(flat_units)
        if nu and phases >= 1:
            emit_consts_w1g0()                 # w1[g0] + xt0 dispatch first
            emit_mm_h(*flat_units[0])
            emit_consts_first()
            if nu > 1:
                emit_mm_h(*flat_units[1])
            for g in range(1, NG):
                nc.sync.dma_start(out=w1_sb[:, g, :], in_=w1_ap[:, g, :])
            for ui in range(nu):
                ci, kt = flat_units[ui]
                if ui + 2 < nu:
                    emit_mm_h(*flat_units[ui + 2])
                ht_sb = emit_relu(ci, kt)
                emit_og_mms(ci, kt, ht_sb)
                # stream group g+1's layer-2 weights during group g's
                # first chunk (well ahead of first use)
                if kt == 0 and chunks[ci][0] == 0 and chunks[ci][1] < NG - 1:
                    emit_consts_w2(chunks[ci][1] + 1)
                # interleave gathers/trees at chunk-set boundaries
                if kt == KT - 1 and chunks[ci][1] == NG - 1:
                    j = chunks[ci][0]
                    if j == 0:
                        emit_consts_late()
                    if phases >= 2:
                        for w in sorted(list(pending_gather)):
                            if ready[w] <= j - 1:
                                emit_tree(w, pending_gather.pop(w))
                                treed.append(w)
                        if phases >= 3:
                            while (p3done < len(treed)
                                   and ready[treed[p3done]] <= j - 2):
                                emit_phase3(treed[p3done])
                                p3done += 1
                        # fire prepared DMAs for windows whose og is ready
                        for w in range(NBLK):
                            if ready[w] == j and w in prepped:
                                emit_gather_trigger(w)
                                pending_gather[w] = prepped.pop(w)
                        # generate descriptors for next boundary's windows
                        for w in range(NBLK):
                            if ready[w] == j + 1:
                                prepped[w] = emit_gather_prep(w)
            for w in sorted(list(pending_gather)):
                emit_tree(w, pending_gather.pop(w))
                treed.append(w)
            if phases >= 3:
                while p3done < len(treed):
                    emit_phase3(treed[p3done])
                    p3done += 1
        else:
            emit_consts_w1g0()
            emit_consts_first()
            for g in range(1, NG):
                nc.sync.dma_start(out=w1_sb[:, g, :], in_=w1_ap[:, g, :])
            for g in range(1, NG):
                emit_consts_w2(g)
            emit_consts_late()
        p1ctx.close()

        if dbg_og_ap is not None:
            nc.sync.dma_start(out=dbg_og_ap[:], in_=og_ap[:])
        if dbg_emb_ap is not None:
            nc.sync.dma_start(out=dbg_emb_ap[:], in_=emb_sb[:])

        # --- phase 4: instance norm (per graph, per channel) ---------------
        embn = [cst.tile([P, GPC, CCAP], F32R, tag=f"embn{f}", name=f"embn{f}")
                for f in range(FT)]
        with tc.tile_pool(name="p4", bufs=8) as p4:
            for f in range(FT if phases >= 4 else 0):
                sm = p4.tile([P, GPC], F32, tag="sm")
                nc.vector.tensor_reduce(sm[:], embt[f][:], mybir.AxisListType.X,
                                        ALU.add)
                sq = p4.tile([P, GPC, CCAP], F32, tag="sq")
                nc.scalar.activation(sq[:], embt[f][:], AF.Square)
                s2 = p4.tile([P, GPC], F32, tag="s2")
                nc.vector.tensor_reduce(s2[:], sq[:], mybir.AxisListType.X,
                                        ALU.add)
                mean = p4.tile([P, GPC], F32, tag="mean")
                nc.vector.tensor_tensor(out=mean[:], in0=sm[:], in1=ic_sb[:],
                                        op=ALU.mult)
                ex2 = p4.tile([P, GPC], F32, tag="ex2")
                nc.vector.tensor_tensor(out=ex2[:], in0=s2[:], in1=ic_sb[:],
                                        op=ALU.mult)
                m2 = p4.tile([P, GPC], F32, tag="m2")
                nc.vector.tensor_tensor(out=m2[:], in0=mean[:], in1=mean[:],
                                        op=ALU.mult)
                var = p4.tile([P, GPC], F32, tag="var")
                nc.vector.tensor_tensor(out=var[:], in0=ex2[:], in1=m2[:],
                                        op=ALU.subtract)
                ve = p4.tile([P, GPC], F32, tag="ve")
                nc.vector.tensor_scalar_add(ve[:], var[:], EPS)
                sd = p4.tile([P, GPC], F32, tag="sd")
                nc.scalar.activation(sd[:], ve[:], AF.Sqrt)
                rstd = p4.tile([P, GPC], F32, tag="rstd")
                nc.vector.reciprocal(rstd[:], sd[:])
                for gi_ in range(GPC):
                    nc.vector.tensor_scalar(
                        embn[f][:, gi_, :], embt[f][:, gi_, :],
                        mean[:, gi_:gi_ + 1], rstd[:, gi_:gi_ + 1],
                        op0=ALU.subtract, op1=ALU.mult)

        # --- phase 5: classifier ------------------------------------------
        NSL = 512
        with tc.tile_pool(name="p5", bufs=2) as p5, \
             tc.tile_pool(name="p5ps", bufs=2, space="PSUM") as p5ps:
            for n0 in (range(0, SLOTS, NSL) if phases >= 5 else []):
                nw = min(NSL, SLOTS - n0)
                lg_ps = p5ps.tile([C_CLS, NSL], F32, tag="lg")
                for f in range(FT):
                    rhs = embn[f].rearrange("p g c -> p (g c)")[:, n0:n0 + nw]
                    nc.tensor.matmul(lg_ps[:, :nw], wo_sb[:, f, :], rhs,
                                     start=(f == 0), stop=(f == FT - 1))
                lg_sb = p5.tile([C_CLS, NSL], F32, tag="lgs")
                nc.vector.tensor_scalar(lg_sb[:, :nw], lg_ps[:, :nw],
                                        bo_sb[:], None, op0=ALU.add)
                nc.sync.dma_start(out=lo_ap[:, n0:n0 + nw], in_=lg_sb[:, :nw])

    nc.compile()
    return nc


# ----------------------------------------------------------------------------
# Host-side sharding / index plumbing
# ----------------------------------------------------------------------------

def _round_up(v, m):
    return (v + m - 1) // m * m


def _pow2_round(v):
    p = 1
    while p < v:
        p *= 2
    return p


def prepare(x, group_indices, pool_cluster_fine, batch_cluster_coarse,
            W1, b1, W2, b2, w_out, b_out):
    """Compute capacities + per-core input maps. Returns (key, in_maps, meta)."""
    x = np.asarray(x)
    gidx = np.asarray(group_indices)
    pcf = np.asarray(pool_cluster_fine).astype(np.int64)
    bcc = np.asarray(batch_cluster_coarse).astype(np.int64)
    W1 = np.asarray(W1, dtype=np.float32)
    b1 = np.asarray(b1, dtype=np.float32)
    W2 = np.asarray(W2, dtype=np.float32)
    b2 = np.asarray(b2, dtype=np.float32)
    w_out = np.asarray(w_out, dtype=np.float32)
    b_out = np.asarray(b_out, dtype=np.float32)

    GPC = G_SEG // NCORES
    KT = KEXP // P

    # node -> group (later groups win on duplicates, matching scatter order)
    gid = np.full(N, -1, np.int32)
    for g in range(NG):
        gid[gidx[g]] = g

    # graph/cluster/node boundaries (general sorted-segment support)
    fine_lo = np.searchsorted(bcc, np.arange(0, G_SEG, GPC))          # per core
    fine_hi = np.searchsorted(bcc, np.arange(GPC - 1, G_SEG, GPC), "right")
    node_lo = np.searchsorted(pcf, fine_lo)
    node_hi = np.searchsorted(pcf, fine_hi)

    cl_lo = np.searchsorted(pcf, np.arange(F_SEG))
    cl_hi = np.searchsorted(pcf, np.arange(F_SEG), "right")
    cl_sz = cl_hi - cl_lo
    MCAP = _pow2_round(max(2, int(cl_sz.max())))

    g_lo = np.searchsorted(bcc, np.arange(G_SEG))
    g_hi = np.searchsorted(bcc, np.arange(G_SEG), "right")
    g_sz = g_hi - g_lo
    CCAP = _round_up(max(1, int(g_sz.max())), P)
    SLOTS = GPC * CCAP
    NBLK = SLOTS // P
    BPG = CCAP // P

    # rows per (core, group)
    counts = np.zeros((NCORES, NG), np.int64)
    core_nodes = []
    for c in range(NCORES):
        nd = np.arange(node_lo[c], node_hi[c])
        core_nodes.append(nd)
        gs = gid[nd]
        for g in range(NG):
            counts[c, g] = int((gs == g).sum())
    GCAP = _round_up(max(1, int(counts.max())), P)
    RTOT = NG * GCAP

    # ---- window margin M: windows are 512-spaced in group-row space; the
    # members of cluster-block w of group g live at rows [off(w), off(w+1)).
    # M must cover |off_g(w) - 512w| for all (core, w, g).
    maxdev = 0
    all_offs = []
    for c in range(NCORES):
        nd = core_nodes[c]
        gs = gid[nd]
        # block boundaries in node space (per core)
        # block t covers clusters [fine_lo[c] + sum of full blocks...]
        # build per-graph cluster slots, blocks of 128 slots never straddle
        # graphs (CCAP % 128 == 0)
        blk_node_lo = []
        for t in range(NBLK):
            gi = t // BPG
            bi = t % BPG
            gg = c * GPC + gi
            c0 = g_lo[gg] + bi * P
            c1 = min(g_lo[gg] + min((bi + 1) * P, int(g_sz[gg])), g_hi[gg])
            if c0 >= g_hi[gg] or c1 <= c0:   # block entirely padding
                blk_node_lo.append(None)
                continue
            blk_node_lo.append((int(cl_lo[c0]), int(cl_hi[c1 - 1])))
        # per-group prefix offsets at block boundaries
        offs = np.zeros((NG, NBLK + 1), np.int64)
        for g in range(NG):
            rows_nodes = nd[gs == g]          # node ids, sorted
            for t in range(NBLK):
                if blk_node_lo[t] is None:
                    offs[g, t + 1] = offs[g, t]
                    continue
                n0, n1 = blk_node_lo[t]
                offs[g, t] = np.searchsorted(rows_nodes, n0)
                offs[g, t + 1] = np.searchsorted(rows_nodes, n1)
            for t in range(NBLK + 1):
                grid = 512 * t
                maxdev = max(maxdev, abs(int(offs[g, t]) - grid))
        all_offs.append(offs)
    Mw = _round_up(max(32, maxdev), 32)
    assert Mw <= 512, f"readiness margin {Mw} too large; input too irregular"
    # every block's rows must be written by its scheduled readiness
    assert int(counts.max()) <= 512 * NBLK + Mw, \
        f"rows {counts.max()} exceed readiness coverage {512 * NBLK + Mw}"

    zero_b2 = bool(np.all(b2 == 0.0))
    OGROWS = 2 + NG * GCAP
    assert OGROWS < 32768, f"og rows {OGROWS} exceed int16 gather range"

    # replicated weight prep (shared across cores)
    bf = ml_dtypes.bfloat16
    w1_h = np.ascontiguousarray(W1.transpose(1, 0, 2)).astype(bf)  # [128,4,1024]
    w2_h = np.ascontiguousarray(
        W2.reshape(NG, KT, P, H).transpose(2, 0, 1, 3)).astype(bf)  # [128,4,8,512]
    b1_h = np.ascontiguousarray(
        b1.reshape(NG, KT, P).transpose(2, 0, 1).reshape(P, -1))    # [128,32]
    b2_h = np.ascontiguousarray(
        np.broadcast_to(b2[None, :, :], (P, NG, H)))                # [128,4,512]
    wo_h = np.ascontiguousarray(
        w_out.reshape(H // P, P, C_CLS).transpose(1, 0, 2))         # [128,4,16]
    bo_h = np.ascontiguousarray(b_out.reshape(C_CLS, 1))            # [16,1]
    id_h = np.eye(P, dtype=bf)

    IDXW = MCAP * P // 16

    in_maps = []
    meta = []
    for c in range(NCORES):
        nd = core_nodes[c]
        gs = gid[nd]
        xt = np.zeros((P, RTOT), bf)
        rowid = np.full(N, -1, np.int64)     # node -> group-row index
        nodeg = np.full(N, -1, np.int8)
        for g in range(NG):
            sel = nd[gs == g]
            cnt = len(sel)
            xt[:, g * GCAP:g * GCAP + cnt] = x[sel].T.astype(bf)
            rowid[sel] = np.arange(cnt)
            nodeg[sel] = g

        inv_cnt = np.zeros(GPC, np.float32)
        for gi in range(GPC):
            gg = c * GPC + gi
            inv_cnt[gi] = 1.0 / max(int(g_sz[gg]), 1)

        # member table: per block t, [128 clusters, MCAP] flat og row ids
        gidx_w = np.zeros((P, NBLK * IDXW), np.int16)
        for t in range(NBLK):
            gi = t // BPG
            bi = t % BPG
            gg = c * GPC + gi
            member = np.ones((P, MCAP), np.int64)    # 1 = -inf sentinel
            for a in range(P):                        # slot within block
                ci_g = bi * P + a                     # cluster idx within graph
                if ci_g >= g_sz[gg]:
                    member[a, :] = 0                  # pad cluster -> zeros row
                    continue
                f = g_lo[gg] + ci_g
                mm = 0
                for n in range(int(cl_lo[f]), int(cl_hi[f])):
                    g = int(nodeg[n])
                    if g < 0:
                        continue                      # node in no group -> 0
                    member[a, mm] = 2 + g * GCAP + int(rowid[n])
                    mm += 1
                # nodes with no group contribute zeros (row 0)
                nz = int(cl_sz[f]) - mm
                for _ in range(nz):
                    member[a, mm] = 0
                    mm += 1
            seq = member.T.reshape(-1)                # i = m*128 + a
            wseq = seq.reshape(-1, 16).T.astype(np.int16)
            gidx_w[:, t * IDXW:(t + 1) * IDXW] = np.tile(wseq, (8, 1))

        in_maps.append({
            "xt": xt,
            "w1": w1_h, "w2": w2_h, "b1s": b1_h, "b2r": b2_h,
            "wout": wo_h, "bout": bo_h,
            "invc": np.broadcast_to(inv_cnt[None, :], (P, GPC)).copy(),
            "ident": id_h,
            "gidx": gidx_w,
        })
        meta.append({"fine_lo": int(fine_lo[c]), "fine_hi": int(fine_hi[c]),
                     "g_lo": g_lo, "g_sz": g_sz, "c": c})

    key = (GCAP, CCAP, MCAP, Mw, zero_b2)
    return key, in_maps, meta, (CCAP,)


def get_runner(key, phases=5):
    ck = (key, phases)
    if ck not in _PROGRAM_CACHE:
        nc = _build_program(*key, phases=phases)
        _PROGRAM_CACHE[ck] = nc
    return _PROGRAM_CACHE[ck]


def kernel(**inputs) -> np.ndarray:
    key, in_maps, meta, (CCAP,) = prepare(**inputs)
    nc = get_runner(key)
    results = bass2jax.run_bass_via_pjrt(nc, in_maps, n_cores=NCORES)

    bcc = np.asarray(inputs["batch_cluster_coarse"]).astype(np.int64)
    GPC = G_SEG // NCORES
    g_lo = np.searchsorted(bcc, np.arange(G_SEG))
    out = np.zeros((F_SEG, C_CLS), np.float32)
    for c in range(NCORES):
        lo = results[c]["logt"]              # [16, SLOTS]
        for gi in range(GPC):
            gg = c * GPC + gi
            sz = int(meta[c]["g_sz"][gg])
            f0 = int(g_lo[gg])
            sl0 = gi * CCAP
            out[f0:f0 + sz, :] = lo[:, sl0:sl0 + sz].T
    return out

